# revision 11
# baseline (speedup 1.0000x reference)
"""BitNet transformer layer on 8 Trainium2 NeuronCores.

Sharding: 2 batches x 4 sequence-slices of 512 tokens (no collectives).
Each core computes K/V for its whole batch (replicated within its 4-core
group) and Q/attention/o_proj/MLP for its own 512 tokens.  Per-core inputs
are np.roll'ed so the core's own tokens come first -> one NEFF for all
cores (attention is permutation-invariant over keys).

Numerics: quantized activations are int8-valued and weights ternary, both
exact in bf16, so every bitlinear matmul is bit-exact (fp32 PSUM accum of
integers).  Attention scores / A / V run in fp32r (~1.6e-4).  Softmax skips
the max-subtraction (args bounded) and gets its denominator free via a
ones-column appended to V.  round() uses the magic-number trick (RNE,
matches jnp.round).
"""
import sys
for _p in ('/opt/trn_rl_repo', '/root/.axon_site/_ro/trn_rl_repo'):
    if _p not in sys.path:
        sys.path.append(_p)

import os
import numpy as np
import ml_dtypes
from contextlib import ExitStack

import concourse.bass as bass
import concourse.tile as tile
from concourse import bacc, mybir
from concourse.bass_utils import run_bass_kernel_spmd
from concourse.masks import make_identity

f32 = np.float32
bf16 = ml_dtypes.bfloat16
FP32 = mybir.dt.float32
FP32R = mybir.dt.float32r
BF16 = mybir.dt.bfloat16
AF = mybir.ActivationFunctionType
ALU = mybir.AluOpType
AX = mybir.AxisListType

P = 128
HID, INTER, NH, NKV, HD = 2560, 6912, 20, 5, 128
KH = HID // P            # 20
KI = INTER // P          # 54
S, T = 2048, 512         # batch seq len, per-core tokens
SC, TC = S // P, T // P  # 16, 4
EPS = 1e-5
MAGIC = float(1.5 * 2 ** 23)
NTI = 14                 # INTER tiles of 512 (padded 6912 -> 7168)
INTER_PAD = NTI * 512

# wsc column indices
IQ, IK, IV, IO, IG, IU, ID = range(7)


# --------------------------------------------------------------------------
# device kernel
# --------------------------------------------------------------------------

def _norm_quant(nc, pool, stat, x_ap, D, y_bf, dq_dst, magic=None):
    """rmsnorm (unit ln weight) + activation_quant of x_ap [128, D] fp32.

    Writes int-valued bf16 quant to y_bf [128, D]; per-token dequant scale
    (max|normed|/127) to dq_dst [128,1].  Scratch from `pool` / `stat`.
    """
    m = stat.tile([P, 1], FP32, tag="nq_m")
    nc.vector.tensor_reduce(m[:], x_ap, AX.X, ALU.max, apply_absolute_value=True)
    sq = pool.tile([P, D], FP32, tag=f"nq_sq{D}")
    ssum = stat.tile([P, 1], FP32, tag="nq_ss")
    nc.scalar.activation(sq[:], x_ap, AF.Square, accum_out=ssum[:])
    v1 = stat.tile([P, 1], FP32, tag="nq_v1")
    nc.vector.tensor_scalar(v1[:], ssum[:], 1.0 / D, EPS, ALU.mult, ALU.add)
    r = stat.tile([P, 1], FP32, tag="nq_r")
    nc.scalar.activation(r[:], v1[:], AF.Sqrt)
    ir0 = stat.tile([P, 1], FP32, tag="nq_ir0")
    nc.vector.reciprocal(ir0[:], r[:])
    # one Newton step for rsqrt: ir = ir0*(1.5 - 0.5*v1*ir0^2)
    t1 = stat.tile([P, 1], FP32, tag="nq_t1")
    nc.vector.tensor_tensor(t1[:], v1[:], ir0[:], ALU.mult)
    nc.vector.tensor_tensor(t1[:], t1[:], ir0[:], ALU.mult)
    nc.vector.tensor_scalar(t1[:], t1[:], -0.5, 1.5, ALU.mult, ALU.add)
    ir = stat.tile([P, 1], FP32, tag="nq_ir")
    nc.vector.tensor_tensor(ir[:], ir0[:], t1[:], ALU.mult)
    # mc = max(m*ir, 1e-5); cc = (1/mc)*ir*127 ; dq = mc/127
    mn = stat.tile([P, 1], FP32, tag="nq_mn")
    nc.vector.tensor_tensor(mn[:], m[:], ir[:], ALU.mult)
    nc.vector.tensor_scalar(mn[:], mn[:], 1e-5, None, ALU.max)
    rc = stat.tile([P, 1], FP32, tag="nq_rc")
    nc.vector.reciprocal(rc[:], mn[:])
    cc = stat.tile([P, 1], FP32, tag="nq_cc")
    nc.vector.tensor_scalar(cc[:], rc[:], ir[:], 127.0, ALU.mult, ALU.mult)
    nc.vector.tensor_scalar(dq_dst, mn[:], 1.0 / 127.0, None, ALU.mult)
    # quant: y = (x*cc + MAGIC) - MAGIC  (RNE round), cast bf16
    rnd = pool.tile([P, D], FP32, tag=f"nq_sq{D}")  # reuse sq slot
    nc.scalar.activation(rnd[:], x_ap, AF.Identity, bias=magic[:], scale=cc[:])
    nc.vector.tensor_scalar(y_bf, rnd[:], -MAGIC, None, ALU.add)


def _rope_evict(nc, pool, ps_ap, cos_sl, sinneg_sl, out_r):
    """out_r [128, n] f32r = ps*cos + rotate_half(ps)*sin (sin rows 0:64
    pre-negated).  ps_ap is a [128, n] psum AP."""
    n = ps_ap.shape[-1]
    ta = pool.tile([P, n], FP32, tag=f"rp_a{n}")
    tb = pool.tile([P, n], FP32, tag=f"rp_b{n}")
    nc.vector.tensor_tensor(ta[:], ps_ap, cos_sl, ALU.mult)
    nc.vector.tensor_tensor(tb[0:64, :], ps_ap[64:128, :], sinneg_sl[0:64, :], ALU.mult)
    nc.vector.tensor_tensor(tb[64:128, :], ps_ap[0:64, :], sinneg_sl[64:128, :], ALU.mult)
    nc.vector.tensor_tensor(out_r, ta[:], tb[:], ALU.add)


def build_nc():
    nc = bacc.Bacc(None, target_bir_lowering=False, debug=False)

    hs = nc.dram_tensor("hs", [S, HID], FP32, kind="ExternalInput")
    cosT = nc.dram_tensor("cosT", [P, S], FP32, kind="ExternalInput")
    sinT = nc.dram_tensor("sinT", [P, S], FP32, kind="ExternalInput")
    wq_c = nc.dram_tensor("wq_c", [NH, P, KH, P], BF16, kind="ExternalInput")
    wk_c = nc.dram_tensor("wk_c", [NKV, P, KH, P], BF16, kind="ExternalInput")
    wv_w = nc.dram_tensor("wv_w", [P, KH, NKV * HD], BF16, kind="ExternalInput")
    wo_c = nc.dram_tensor("wo_c", [5, P, KH, 512], BF16, kind="ExternalInput")
    wg_c = nc.dram_tensor("wg_c", [NTI, P, KH, 512], BF16, kind="ExternalInput")
    wu_c = nc.dram_tensor("wu_c", [NTI, P, KH, 512], BF16, kind="ExternalInput")
    wd_c = nc.dram_tensor("wd_c", [5, 2, P, 27, 512], BF16, kind="ExternalInput")
    wsc = nc.dram_tensor("wsc", [P, 8], FP32, kind="ExternalInput")
    out = nc.dram_tensor("out", [T, HID], FP32, kind="ExternalOutput")
    KDEBUG = os.environ.get("KDEBUG", "0") == "1"
    if KDEBUG:
        d_qxT = nc.dram_tensor("d_qxT", [P, KH, S], BF16, kind="ExternalOutput")
        d_dq = nc.dram_tensor("d_dq", [P, SC], FP32, kind="ExternalOutput")
        d_kT = nc.dram_tensor("d_kT", [NKV, P, S], FP32, kind="ExternalOutput")
        d_qT = nc.dram_tensor("d_qT", [NH, P, T], FP32, kind="ExternalOutput")
        d_v = nc.dram_tensor("d_v", [NKV, P, SC * 160], FP32, kind="ExternalOutput")
        d_ao = nc.dram_tensor("d_ao", [T, HID], FP32, kind="ExternalOutput")
        d_h2 = nc.dram_tensor("d_h2", [T, HID], FP32, kind="ExternalOutput")
        d_gu = nc.dram_tensor("d_gu", [T, INTER_PAD], FP32, kind="ExternalOutput")

    with tile.TileContext(nc) as tc, ExitStack() as top:
        dram = top.enter_context(tc.tile_pool(name="dram", bufs=1, space="DRAM"))
        kT_d = dram.tile([NKV, P, S], FP32R)
        qT_d = dram.tile([NH, P, T], FP32R)
        v_d = dram.tile([NKV, P, SC * 160], FP32R)
        ao_d = dram.tile([T, HID], FP32)
        h2_d = dram.tile([T, HID], FP32)
        gu_d = dram.tile([T, INTER_PAD], FP32)

        const = top.enter_context(tc.tile_pool(name="const", bufs=1))
        stat = top.enter_context(tc.tile_pool(name="stat", bufs=3))
        wsc_sb = const.tile([P, 8], FP32)
        nc.sync.dma_start(wsc_sb[:], wsc[:])
        ident = const.tile([P, P], FP32)
        make_identity(nc, ident[:])
        magic_sb = const.tile([P, 1], FP32)
        nc.vector.memset(magic_sb[:], MAGIC)

        # ============================ phase B =============================
        bres_ctx = ExitStack()
        bres = bres_ctx.enter_context(tc.tile_pool(name="bres", bufs=1))
        # B1: norm+quant+transpose of all S tokens -> qxT resident
        qxT = bres.tile([P, KH, S], BF16)           # 80KB/part
        dq_all = bres.tile([P, SC], FP32)
        with tc.tile_pool(name="b1", bufs=2) as b1:
            for sc in range(SC):
                x = b1.tile([P, HID], FP32, tag="b1x")
                nc.sync.dma_start(x[:], hs[sc * P:(sc + 1) * P, :])
                y = b1.tile([P, HID], BF16, tag="b1y")
                _norm_quant(nc, b1, stat, x[:], HID, y[:], dq_all[:, sc:sc + 1], magic_sb)
                nc.sync.dma_start_transpose(qxT[:, :, sc * P:(sc + 1) * P], y[:])

        # dq_row [1, S] via PE transpose of dq_all
        dq_row = bres.tile([1, S], FP32)
        with tc.tile_pool(name="b2", bufs=1) as b2, \
             tc.tile_pool(name="b2p", bufs=1, space="PSUM") as b2p:
            dqt_ps = b2p.tile([SC, P], FP32)
            nc.tensor.transpose(dqt_ps[:], dq_all[:], ident[:])
            dqt = b2.tile([SC, P], FP32)
            nc.scalar.copy(dqt[:], dqt_ps[:])
            nc.sync.dma_start(dq_row[:].rearrange("o (s p) -> o s p", p=P), dqt[:])

        # rope tables with folded dequant scales
        cosk = bres.tile([P, S], FP32)
        sinkn = bres.tile([P, S], FP32)
        cosq = bres.tile([P, T], FP32)
        sinqn = bres.tile([P, T], FP32)
        with tc.tile_pool(name="b3", bufs=1) as b3:
            ct = b3.tile([P, S], FP32, tag="b3c")
            st_ = b3.tile([P, S], FP32, tag="b3s")
            nc.sync.dma_start(ct[:], cosT[:])
            nc.sync.dma_start(st_[:], sinT[:])
            dq_bc = b3.tile([P, S], FP32, tag="b3bc")
            nc.gpsimd.partition_broadcast(dq_bc[:], dq_row[:])
            nc.vector.tensor_tensor(cosk[:], ct[:], dq_bc[:], ALU.mult)
            nc.vector.tensor_scalar(cosk[:], cosk[:], wsc_sb[:, IK:IK + 1], None, ALU.mult)
            nc.vector.tensor_tensor(sinkn[:], st_[:], dq_bc[:], ALU.mult)
            nc.vector.tensor_scalar(sinkn[0:64, :], sinkn[0:64, :], wsc_sb[0:64, IK:IK + 1], None, ALU.mult)
            nc.vector.tensor_scalar(sinkn[0:64, :], sinkn[0:64, :], -1.0, None, ALU.mult)
            nc.vector.tensor_scalar(sinkn[64:128, :], sinkn[64:128, :], wsc_sb[64:128, IK:IK + 1], None, ALU.mult)
            nc.vector.tensor_tensor(cosq[:], ct[:, 0:T], dq_bc[:, 0:T], ALU.mult)
            nc.vector.tensor_scalar(cosq[:], cosq[:], wsc_sb[:, IQ:IQ + 1], None, ALU.mult)
            nc.vector.tensor_tensor(sinqn[:], st_[:, 0:T], dq_bc[:, 0:T], ALU.mult)
            nc.vector.tensor_scalar(sinqn[0:64, :], sinqn[0:64, :], wsc_sb[0:64, IQ:IQ + 1], None, ALU.mult)
            nc.vector.tensor_scalar(sinqn[0:64, :], sinqn[0:64, :], -1.0, None, ALU.mult)
            nc.vector.tensor_scalar(sinqn[64:128, :], sinqn[64:128, :], wsc_sb[64:128, IQ:IQ + 1], None, ALU.mult)

        # B-v: V projection (natural orient), evict f32r with ones column
        with tc.tile_pool(name="bv", bufs=2) as bv, \
             tc.tile_pool(name="bvw", bufs=1) as bvw, \
             tc.tile_pool(name="bvp", bufs=2, space="PSUM") as bvp:
            wv_sb = bvw.tile([P, KH, NKV * HD], BF16)
            nc.sync.dma_start(wv_sb[:], wv_w[:])
            for sc in range(SC):
                ps = bvp.tile([P, NKV * HD], FP32, tag="vps")
                for kc in range(KH):
                    nc.tensor.matmul(ps[:, 0:512], lhsT=qxT[:, kc, sc * P:(sc + 1) * P],
                                     rhs=wv_sb[:, kc, 0:512],
                                     start=(kc == 0), stop=(kc == KH - 1))
                for kc in range(KH):
                    nc.tensor.matmul(ps[:, 512:640], lhsT=qxT[:, kc, sc * P:(sc + 1) * P],
                                     rhs=wv_sb[:, kc, 512:640],
                                     start=(kc == 0), stop=(kc == KH - 1))
                stg = bv.tile([P, NKV, 160], FP32R, tag="vstg")
                nc.vector.memset(stg[:, :, 128:160].bitcast(FP32), 0.0)
                nc.scalar.activation(stg[:, :, 0:128],
                                     ps[:].rearrange("p (h f) -> p h f", f=128),
                                     AF.Copy, scale=dq_all[:, sc:sc + 1])
                nc.vector.memset(stg[:, :, 128:129].bitcast(FP32), 1.0)
                nc.sync.dma_start(
                    v_d[:, :, sc * 160:(sc + 1) * 160].rearrange("h p f -> p h f"),
                    stg[:])

        # B-k: K projection (swapped orient) + rope -> kT_d
        with tc.tile_pool(name="bk", bufs=2) as bk, \
             tc.tile_pool(name="bkp", bufs=2, space="PSUM") as bkp:
            for oc in range(NKV):
                wchunk = bk.tile([P, KH, P], BF16, tag="kwc")
                nc.sync.dma_start(wchunk[:], wk_c[oc])
                for ts in range(4):
                    ps = bkp.tile([P, 512], FP32, tag="kps")
                    for kc in range(KH):
                        nc.tensor.matmul(ps[:], lhsT=wchunk[:, kc, :],
                                         rhs=qxT[:, kc, ts * 512:(ts + 1) * 512],
                                         start=(kc == 0), stop=(kc == KH - 1))
                    ko = bk.tile([P, 512], FP32R, tag="kout")
                    _rope_evict(nc, bk, ps[:], cosk[:, ts * 512:(ts + 1) * 512],
                                sinkn[:, ts * 512:(ts + 1) * 512], ko[:])
                    nc.sync.dma_start(kT_d[oc, :, ts * 512:(ts + 1) * 512], ko[:])

        # B-q: Q projection (own 512 tokens) + rope -> qT_d
        with tc.tile_pool(name="bq", bufs=2) as bq, \
             tc.tile_pool(name="bqp", bufs=2, space="PSUM") as bqp:
            for oc in range(NH):
                wchunk = bq.tile([P, KH, P], BF16, tag="qwc")
                nc.sync.dma_start(wchunk[:], wq_c[oc])
                ps = bqp.tile([P, T], FP32, tag="qps")
                for kc in range(KH):
                    nc.tensor.matmul(ps[:], lhsT=wchunk[:, kc, :],
                                     rhs=qxT[:, kc, 0:T],
                                     start=(kc == 0), stop=(kc == KH - 1))
                qo = bq.tile([P, T], FP32R, tag="qout")
                _rope_evict(nc, bq, ps[:], cosq[:], sinqn[:], qo[:])
                nc.sync.dma_start(qT_d[oc], qo[:])

        if KDEBUG:
            nc.sync.dma_start(d_qxT[:], qxT[:])
            nc.sync.dma_start(d_dq[:], dq_all[:])
        bres_ctx.close()
        if KDEBUG:
            nc.sync.dma_start(d_kT[:], kT_d[:].bitcast(FP32))
            nc.sync.dma_start(d_qT[:], qT_d[:].bitcast(FP32))
            nc.sync.dma_start(d_v[:], v_d[:].bitcast(FP32))

        # ============================ phase C =============================
        with tc.tile_pool(name="ckv", bufs=2) as ckv, \
             tc.tile_pool(name="cat", bufs=3) as cat, \
             tc.tile_pool(name="cst", bufs=2, space="PSUM") as cst, \
             tc.tile_pool(name="cav", bufs=1, space="PSUM") as cav:
            for kv in range(NKV):
                kT = ckv.tile([P, S], FP32R, tag="ckT")
                nc.sync.dma_start(kT[:], kT_d[kv])
                vsb = ckv.tile([P, 17 * 160], FP32R, tag="cv")
                nc.sync.dma_start(vsb[:, 0:SC * 160], v_d[kv])
                nc.vector.memset(vsb[:, SC * 160:].bitcast(FP32), 0.0)
                for r in range(4):
                    h = kv * 4 + r
                    qT = cat.tile([P, T], FP32R, tag="cqT")
                    nc.sync.dma_start(qT[:], qT_d[h])
                    avs = [cav.tile([P, 256], FP32, tag=f"av{qs}", name=f"av{qs}")
                           for qs in range(4)]
                    for g in range(8):
                        stp = cst.tile([P, 1024], FP32, tag="cstp")
                        for j in range(2):
                            ktc = g * 2 + j
                            nc.tensor.matmul(stp[:, j * 512:(j + 1) * 512],
                                             lhsT=kT[:, ktc * P:(ktc + 1) * P],
                                             rhs=qT[:], start=True, stop=True)
                        at = cat.tile([P, 1024], FP32R, tag="cA")
                        nc.scalar.activation(at[:], stp[:], AF.Exp,
                                             bias=0.0, scale=float(HD ** -0.5))
                        for j in range(2):
                            ktc = g * 2 + j
                            for qs in range(4):
                                nc.tensor.matmul(
                                    avs[qs][:],
                                    lhsT=at[:, j * 512 + qs * P: j * 512 + (qs + 1) * P],
                                    rhs=vsb[:, ktc * 160: ktc * 160 + 256],
                                    start=(ktc == 0), stop=(ktc == SC - 1))
                    for qs in range(4):
                        den = stat.tile([P, 1], FP32, tag="cden")
                        nc.vector.reciprocal(den[:], avs[qs][:, 128:129])
                        nc.vector.tensor_scalar(den[:], den[:], wsc_sb[:, IV:IV + 1],
                                                None, ALU.mult)
                        ao = cat.tile([P, HD], FP32, tag="cao")
                        nc.scalar.activation(ao[:], avs[qs][:, 0:HD], AF.Copy,
                                             scale=den[:])
                        nc.sync.dma_start(
                            ao_d[qs * P:(qs + 1) * P, h * HD:(h + 1) * HD], ao[:])

        if KDEBUG:
            nc.sync.dma_start(d_ao[:], ao_d[:])

        # ============================ phase D =============================
        with tc.tile_pool(name="dn", bufs=2) as dn, \
             tc.tile_pool(name="dw", bufs=2) as dw, \
             tc.tile_pool(name="dp", bufs=3, space="PSUM") as dp:
            oxT = dn.tile([P, KH, T], BF16, tag="oxT")
            dqo = stat.tile([P, TC], FP32, tag="dqo")
            for ts in range(TC):
                x = dn.tile([P, HID], FP32, tag="dx")
                nc.sync.dma_start(x[:], ao_d[ts * P:(ts + 1) * P, :])
                y = dn.tile([P, HID], BF16, tag="dy")
                _norm_quant(nc, dn, stat, x[:], HID, y[:], dqo[:, ts:ts + 1], magic_sb)
                nc.sync.dma_start_transpose(oxT[:, :, ts * P:(ts + 1) * P], y[:])
            dqo_s = stat.tile([P, TC], FP32, tag="dqos")
            nc.vector.tensor_scalar(dqo_s[:], dqo[:], wsc_sb[:, IO:IO + 1], None, ALU.mult)
            for nt in range(5):
                wt = dw.tile([P, KH, 512], BF16, tag="dwo")
                nc.sync.dma_start(wt[:], wo_c[nt])
                for ts in range(TC):
                    ps = dp.tile([P, 512], FP32, tag="dps")
                    for kc in range(KH):
                        nc.tensor.matmul(ps[:], lhsT=oxT[:, kc, ts * P:(ts + 1) * P],
                                         rhs=wt[:, kc, :],
                                         start=(kc == 0), stop=(kc == KH - 1))
                    tmp = dw.tile([P, 512], FP32, tag="dtmp")
                    nc.scalar.activation(tmp[:], ps[:], AF.Copy, scale=dqo_s[:, ts:ts + 1])
                    res = dw.tile([P, 512], FP32, tag="dres")
                    nc.sync.dma_start(res[:], hs[ts * P:(ts + 1) * P,
                                                 nt * 512:(nt + 1) * 512])
                    h2t = dw.tile([P, 512], FP32, tag="dh2")
                    nc.vector.tensor_tensor(h2t[:], tmp[:], res[:], ALU.add)
                    nc.sync.dma_start(h2_d[ts * P:(ts + 1) * P,
                                           nt * 512:(nt + 1) * 512], h2t[:])

        # ============================ phase E =============================
        with tc.tile_pool(name="en", bufs=2) as en, \
             tc.tile_pool(name="ew", bufs=2) as ew, \
             tc.tile_pool(name="ep", bufs=2, space="PSUM") as ep:
            xT = en.tile([P, KH, T], BF16, tag="xT")
            dqm = stat.tile([P, TC], FP32, tag="dqm")
            for ts in range(TC):
                x = en.tile([P, HID], FP32, tag="ex")
                nc.sync.dma_start(x[:], h2_d[ts * P:(ts + 1) * P, :])
                y = en.tile([P, HID], BF16, tag="ey")
                _norm_quant(nc, en, stat, x[:], HID, y[:], dqm[:, ts:ts + 1], magic_sb)
                nc.sync.dma_start_transpose(xT[:, :, ts * P:(ts + 1) * P], y[:])
            sg = stat.tile([P, TC], FP32, tag="sg")
            su = stat.tile([P, TC], FP32, tag="su")
            nc.vector.tensor_scalar(sg[:], dqm[:], wsc_sb[:, IG:IG + 1], None, ALU.mult)
            nc.vector.tensor_scalar(su[:], dqm[:], wsc_sb[:, IU:IU + 1], None, ALU.mult)
            for nt in range(NTI):
                wg_sb = ew.tile([P, KH, 512], BF16, tag="ewg")
                nc.sync.dma_start(wg_sb[:], wg_c[nt])
                wu_sb = ew.tile([P, KH, 512], BF16, tag="ewu")
                nc.sync.dma_start(wu_sb[:], wu_c[nt])
                for ts in range(TC):
                    pg = ep.tile([P, 512], FP32, tag="epg")
                    pu = ep.tile([P, 512], FP32, tag="epu")
                    for kc in range(KH):
                        nc.tensor.matmul(pg[:], lhsT=xT[:, kc, ts * P:(ts + 1) * P],
                                         rhs=wg_sb[:, kc, :],
                                         start=(kc == 0), stop=(kc == KH - 1))
                        nc.tensor.matmul(pu[:], lhsT=xT[:, kc, ts * P:(ts + 1) * P],
                                         rhs=wu_sb[:, kc, :],
                                         start=(kc == 0), stop=(kc == KH - 1))
                    rg = ew.tile([P, 512], FP32, tag="erg")
                    nc.scalar.activation(rg[:], pg[:], AF.Relu, scale=sg[:, ts:ts + 1])
                    ru = ew.tile([P, 512], FP32, tag="eru")
                    nc.scalar.activation(ru[:], pu[:], AF.Copy, scale=su[:, ts:ts + 1])
                    nc.vector.tensor_tensor(rg[:], rg[:], rg[:], ALU.mult)
                    nc.vector.tensor_tensor(rg[:], rg[:], ru[:], ALU.mult)
                    nc.sync.dma_start(gu_d[ts * P:(ts + 1) * P,
                                           nt * 512:(nt + 1) * 512], rg[:])

        if KDEBUG:
            nc.sync.dma_start(d_h2[:], h2_d[:])
            nc.sync.dma_start(d_gu[:], gu_d[:])

        # ============================ phase F =============================
        f_ctx = ExitStack()
        fx2 = f_ctx.enter_context(tc.tile_pool(name="fx2", bufs=1))
        fp = f_ctx.enter_context(tc.tile_pool(name="fp", bufs=3, space="PSUM"))
        if True:
            xT2 = fx2.tile([P, KI, T], BF16, tag="xT2")
            dqf_s = stat.tile([P, TC], FP32, tag="dqfs")
            with tc.tile_pool(name="fn", bufs=1) as fn:
                dqf = stat.tile([P, TC], FP32, tag="dqf")
                for ts in range(TC):
                    x = fn.tile([P, INTER], FP32, tag="fx")
                    nc.sync.dma_start(x[:], gu_d[ts * P:(ts + 1) * P, 0:INTER])
                    y = fn.tile([P, INTER], BF16, tag="fy")
                    _norm_quant(nc, fn, stat, x[:], INTER, y[:], dqf[:, ts:ts + 1], magic_sb)
                    nc.sync.dma_start_transpose(xT2[:, :, ts * P:(ts + 1) * P], y[:])
                nc.vector.tensor_scalar(dqf_s[:], dqf[:], wsc_sb[:, ID:ID + 1],
                                        None, ALU.mult)
            fw = f_ctx.enter_context(tc.tile_pool(name="fw", bufs=2))
            for nt in range(5):
                wd0 = fw.tile([P, 27, 512], BF16, tag="fwd")
                nc.sync.dma_start(wd0[:], wd_c[nt, 0])
                wd1 = fw.tile([P, 27, 512], BF16, tag="fwd")
                nc.sync.dma_start(wd1[:], wd_c[nt, 1])
                for ts in range(TC):
                    ps = fp.tile([P, 512], FP32, tag="fps")
                    for kc in range(KI):
                        w = wd0 if kc < 27 else wd1
                        nc.tensor.matmul(ps[:], lhsT=xT2[:, kc, ts * P:(ts + 1) * P],
                                         rhs=w[:, kc % 27, :],
                                         start=(kc == 0), stop=(kc == KI - 1))
                    tmp = fw.tile([P, 512], FP32, tag="ftmp")
                    nc.scalar.activation(tmp[:], ps[:], AF.Copy, scale=dqf_s[:, ts:ts + 1])
                    res = fw.tile([P, 512], FP32, tag="fres")
                    nc.sync.dma_start(res[:], h2_d[ts * P:(ts + 1) * P,
                                                   nt * 512:(nt + 1) * 512])
                    ot = fw.tile([P, 512], FP32, tag="fot")
                    nc.vector.tensor_tensor(ot[:], tmp[:], res[:], ALU.add)
                    nc.sync.dma_start(out[ts * P:(ts + 1) * P,
                                          nt * 512:(nt + 1) * 512], ot[:])
        f_ctx.close()

    nc.finalize()
    return nc


# --------------------------------------------------------------------------
# host side
# --------------------------------------------------------------------------

def _weight_quant(w):
    w = np.asarray(w, f32)
    mean_abs = np.mean(np.abs(w), dtype=f32).astype(f32)
    wsc = np.maximum(mean_abs, f32(1e-5))
    scale = (f32(1.0) / wsc).astype(f32)
    t = np.clip(np.round(w * scale), -1.0, 1.0).astype(f32)
    return t, float(wsc)


def _kxn(t):
    """t [out,in] ternary -> [128, K/128, out] bf16 (k = ko*128 + p)."""
    K = t.shape[1]
    return np.ascontiguousarray(
        t.T.reshape(K // P, P, t.shape[0]).transpose(1, 0, 2).astype(bf16))


_CACHE = {}


def _get_nc():
    if "nc" not in _CACHE:
        _CACHE["nc"] = build_nc()
    return _CACHE["nc"]


def _prep_weights(inputs):
    ws = {}
    scs = np.zeros(8, f32)
    for i, (nm, key) in enumerate([(IQ, 'wq'), (IK, 'wk'), (IV, 'wv'), (IO, 'wo'),
                                   (IG, 'w_gate'), (IU, 'w_up'), (ID, 'w_down')]):
        t, wsc = _weight_quant(inputs[key])
        ws[key] = t
        scs[nm] = wsc
    a = _kxn(ws['wq'])                       # [128, 20, 2560]
    wq_c = np.ascontiguousarray(
        a.reshape(P, KH, NH, P).transpose(2, 0, 1, 3))
    a = _kxn(ws['wk'])                       # [128, 20, 640]
    wk_c = np.ascontiguousarray(
        a.reshape(P, KH, NKV, P).transpose(2, 0, 1, 3))
    wv_w = _kxn(ws['wv'])                    # [128, 20, 640]
    a = _kxn(ws['wo'])                       # [128, 20, 2560]
    wo_c = np.ascontiguousarray(
        a.reshape(P, KH, 5, 512).transpose(2, 0, 1, 3))
    ag = np.zeros((P, KH, INTER_PAD), bf16)
    ag[:, :, 0:INTER] = _kxn(ws['w_gate'])
    wg_c = np.ascontiguousarray(ag.reshape(P, KH, NTI, 512).transpose(2, 0, 1, 3))
    au = np.zeros((P, KH, INTER_PAD), bf16)
    au[:, :, 0:INTER] = _kxn(ws['w_up'])
    wu_c = np.ascontiguousarray(au.reshape(P, KH, NTI, 512).transpose(2, 0, 1, 3))
    a = _kxn(ws['w_down'])                   # [128, 54, 2560]
    wd_c = np.ascontiguousarray(
        a.reshape(P, 2, 27, 5, 512).transpose(3, 1, 0, 2, 4))
    wsc_arr = np.ascontiguousarray(np.tile(scs[None, :], (P, 1)).astype(f32))
    return dict(wq_c=wq_c, wk_c=wk_c, wv_w=wv_w, wo_c=wo_c,
                wg_c=wg_c, wu_c=wu_c, wd_c=wd_c, wsc=wsc_arr)


def kernel(hidden_states, position_ids, cos, sin, w_in_ln, wq, wk, wv, wo,
           w_attn_sub, w_post_ln, w_gate, w_up, w_down, w_ffn_sub):
    hidden_states = np.asarray(hidden_states, f32)
    cos = np.asarray(cos, f32)
    sin = np.asarray(sin, f32)
    for wname, wv_ in [('w_in_ln', w_in_ln), ('w_attn_sub', w_attn_sub),
                       ('w_post_ln', w_post_ln), ('w_ffn_sub', w_ffn_sub)]:
        assert np.all(np.asarray(wv_) == 1.0), \
            f"{wname}: non-unit rmsnorm weights not supported by this kernel"

    wmaps = _prep_weights(dict(wq=wq, wk=wk, wv=wv, wo=wo, w_gate=w_gate,
                               w_up=w_up, w_down=w_down))
    B = hidden_states.shape[0]
    in_maps = []
    for c in range(8):
        b, sl = c // 4, c % 4
        t0 = sl * T
        m = dict(wmaps)
        m['hs'] = np.ascontiguousarray(np.roll(hidden_states[b], -t0, axis=0))
        m['cosT'] = np.ascontiguousarray(np.roll(cos[b].T.astype(f32), -t0, axis=1))
        m['sinT'] = np.ascontiguousarray(np.roll(sin[b].T.astype(f32), -t0, axis=1))
        in_maps.append(m)

    nc = _get_nc()
    res = run_bass_kernel_spmd(nc, in_maps, core_ids=list(range(8)))
    outp = np.empty((B, S, HID), f32)
    for c in range(8):
        b, sl = c // 4, c % 4
        outp[b, sl * T:(sl + 1) * T] = res.results[c]["out"]
    return outp
